# revision 1
# baseline (speedup 1.0000x reference)
"""Bahdanau-attention kernel for 8 Trainium2 NeuronCores (SPMD, batch-sharded).

Algorithm: scores[t,s] = sum_h v_h * tanh(D[h,t] + E[h,s]) is computed via a
free-frequency sine expansion  tanh(x) ~= sum_k b_k sin(w_k x), which factors
through the angle-addition formula into 2F PSUM-accumulating matmuls over
sin/cos features of D and E separately. Range reduction (sin LUT domain is
~±3.55) uses the f32 round-to-nearest magic-constant trick on DVE; cos
features use cos(2πr) = sin(π/2 − 2π|r|). Softmax runs without max-shift
(scores are bounded, exp cannot overflow); the encoder padding mask is added
as -1e30 into PSUM via a K=1 rank-1 matmul so exp's fused accum_out yields
the row sums directly. The decoder mask folds into the 1/sum scale.
"""
import os
import sys

import numpy as np

if "/opt/trn_rl_repo" not in sys.path:
    sys.path.insert(0, "/opt/trn_rl_repo")

S, T, B, H = 512, 256, 8, 128
F = 6
OMEGA = np.array(
    [0.25538027, 0.76919336, 1.29075108, 1.82168705,
     2.35985974, 2.89107375], dtype=np.float64
)
BK = np.array(
    [1.24329172, 0.34079704, 0.14348461, 0.0587333,
     0.02443713, 0.01325319], dtype=np.float64
)
MAGIC = float(1.5 * 2**23)
TWO_PI = float(2.0 * np.pi)
HALF_PI = float(0.5 * np.pi)
NEG_BIG = -1.0e30

_CACHE = {}
LAST_EXEC_NS = None


def _try_install_trace_hook():
    """Best-effort NTFF profile hook for axon (used only when tracing)."""
    try:
        import contextlib
        import ctypes
        import types

        if "antenv.axon_hooks" in sys.modules:
            return
        lib = ctypes.CDLL("/opt/axon/libaxon_pjrt.so")
        if not hasattr(lib, "axon_start_nrt_profile"):
            return
        lib.axon_start_nrt_profile.argtypes = [
            ctypes.POINTER(ctypes.c_int64),
            ctypes.c_size_t,
        ]
        lib.axon_start_nrt_profile.restype = ctypes.c_int64
        lib.axon_stop_nrt_profile.argtypes = [ctypes.c_char_p]
        lib.axon_stop_nrt_profile.restype = ctypes.c_int64

        @contextlib.contextmanager
        def _hook(output_dir, device_ids):
            import jax

            jax.devices()
            if device_ids:
                ids = (ctypes.c_int64 * len(device_ids))(*device_ids)
                rc = lib.axon_start_nrt_profile(ids, len(device_ids))
            else:
                rc = lib.axon_start_nrt_profile(None, 0)
            if rc != 0:
                raise RuntimeError(f"axon_start_nrt_profile rc={rc}")
            try:
                yield
            finally:
                n = lib.axon_stop_nrt_profile(str(output_dir).encode())
                if n < 0:
                    raise RuntimeError(f"axon_stop_nrt_profile rc={n}")

        mod = types.ModuleType("antenv.axon_hooks")
        _h = _hook

        def set_axon_ntff_profile_hook(h):
            pass

        def get_axon_ntff_profile_hook():
            return _h

        mod.set_axon_ntff_profile_hook = set_axon_ntff_profile_hook
        mod.get_axon_ntff_profile_hook = get_axon_ntff_profile_hook
        sys.modules["antenv.axon_hooks"] = mod
        import antenv

        antenv.axon_hooks = mod
    except Exception:
        pass


def _build():
    if "nc" in _CACHE:
        return _CACHE["nc"]
    import concourse.bacc as bacc
    import concourse.tile as tile
    import concourse.mybir as mybir

    F32 = mybir.dt.float32
    F32R = mybir.dt.float32r
    BF16 = mybir.dt.bfloat16
    AF = mybir.ActivationFunctionType
    AL = mybir.AluOpType

    # k groups, last group smallest for a short pipeline tail; all psum
    # group tiles fit (128, 1536) so 2 slots + 2 score banks = 8 banks
    EG = [[0, 1, 2], [3, 4], [5]]
    DG = [[0, 1, 2, 3], [4, 5]]

    nc = bacc.Bacc("TRN2", target_bir_lowering=False, debug=False, num_devices=8)

    enc_d = nc.dram_tensor("enc", [S, H], F32, kind="ExternalInput")
    dec_d = nc.dram_tensor("dec", [T, H], F32, kind="ExternalInput")
    wd_d = nc.dram_tensor("Wd", [F, H, H], F32R, kind="ExternalInput")
    we_d = nc.dram_tensor("We", [F, H, H], F32R, kind="ExternalInput")
    vbt_d = nc.dram_tensor("vbt", [H, F * T], BF16, kind="ExternalInput")
    eye_d = nc.dram_tensor("eye", [H, H], F32, kind="ExternalInput")
    em_d = nc.dram_tensor("encmask", [1, S], F32, kind="ExternalInput")
    dm_d = nc.dram_tensor("decmask", [T, 1], F32, kind="ExternalInput")
    out_d = nc.dram_tensor("out", [T, S], F32, kind="ExternalOutput")

    with tile.TileContext(nc) as tc:
        with (
            tc.tile_pool(name="cst", bufs=1) as cst,
            tc.tile_pool(name="wrk", bufs=1) as wrk,
            tc.tile_pool(name="ps", bufs=2, space="PSUM") as psp,
        ):
            # ---- inputs to SBUF (consolidated DMAs; critical path first) ----
            with nc.named_scope("dma_in"):
                eye_sb = cst.tile([H, H], F32)
                nc.sync.dma_start(eye_sb[:], eye_d[:])
                enc_sb = cst.tile([128, 4 * H], F32)
                nc.sync.dma_start(
                    enc_sb[:].rearrange("p (c h) -> p c h", c=4),
                    enc_d[:].rearrange("(c p) h -> p c h", p=128),
                )
                dec_sb = cst.tile([128, 2 * H], F32)
                nc.sync.dma_start(
                    dec_sb[:].rearrange("p (c h) -> p c h", c=2),
                    dec_d[:].rearrange("(c p) h -> p c h", p=128),
                )
                wd_sb = cst.tile([H, F * H], F32R)
                nc.sync.dma_start(
                    wd_sb[:].rearrange("p (f h) -> p f h", f=F),
                    wd_d[:].rearrange("f p h -> p f h"),
                )
                we_sb = cst.tile([H, F * H], F32R)
                nc.sync.dma_start(
                    we_sb[:].rearrange("p (f h) -> p f h", f=F),
                    we_d[:].rearrange("f p h -> p f h"),
                )
                vbt_sb = cst.tile([H, F * T], BF16)
                nc.sync.dma_start(vbt_sb[:], vbt_d[:])
                em_sb = cst.tile([1, S], F32)
                nc.sync.dma_start(em_sb[:], em_d[:])
                dm_sb = cst.tile([128, 2], F32)
                nc.sync.dma_start(
                    dm_sb[:],
                    dm_d[:].rearrange("(c p) o -> p (c o)", p=128),
                )

            ones_sb = cst.tile([1, 128], F32)
            nc.vector.memset(ones_sb[:], 1.0)
            hp_sb = cst.tile([128, 1], F32)
            nc.vector.memset(hp_sb[:], HALF_PI)

            # ---- transpose enc/dec to (H, S)/(H, T), cast to f32r ----
            with nc.named_scope("transpose"):
                tr_ps = psp.tile([128, 768], F32, tag="ps")
                for c in range(2):
                    nc.tensor.matmul(
                        tr_ps[:, 512 + c * 128:512 + (c + 1) * 128], dec_sb[:, c * H:(c + 1) * H],
                        eye_sb[:], is_transpose=True, start=True, stop=True,
                    )
                for c in range(4):
                    nc.tensor.matmul(
                        tr_ps[:, c * 128:(c + 1) * 128], enc_sb[:, c * H:(c + 1) * H],
                        eye_sb[:], is_transpose=True, start=True, stop=True,
                    )
                decT = cst.tile([H, T], F32R)
                nc.scalar.copy(decT[:], tr_ps[:, 512:768])
                encT = cst.tile([H, S], F32R)
                nc.scalar.copy(encT[:], tr_ps[:, 0:512])

            args_d = wrk.tile([128, F * T], F32)
            abs_d = wrk.tile([128, F * T], F32)
            args_e = wrk.tile([128, F * S], F32)
            abs_e = wrk.tile([128, F * S], F32)
            fSd = wrk.tile([128, F * T], BF16)
            fCd = wrk.tile([128, F * T], BF16)
            fSe = wrk.tile([128, F * S], BF16)
            fCe = wrk.tile([128, F * S], BF16)

            # ---- pipelines, emission-ordered for engine programs:
            # DVE: rd0, re0, vfold0, re1, rd1, vfold1, re2
            # ACT: d0, e0, e1, d1, e2   PE: ud, ue0, ue1, ue2, masks, scores
            def d_group(g, ks):
                w = len(ks) * T
                gsl = slice(ks[0] * T, ks[0] * T + w)
                with nc.named_scope(f"ud_mm_{g}"):
                    ud_ps = psp.tile([128, 1536], F32, tag="ps", name=f"ud{g}")
                    for kk, k in enumerate(ks):
                        nc.tensor.matmul(
                            ud_ps[:, kk * T:(kk + 1) * T],
                            wd_sb[:, k * H:(k + 1) * H],
                            decT[:],
                            start=True,
                            stop=True,
                        )
                with nc.named_scope(f"round_d_{g}"):
                    i_d = wrk.tile([128, 1024], F32, tag="i_d", name=f"id{g}")
                    nc.vector.tensor_scalar(i_d[:, 0:w], ud_ps[:, 0:w], MAGIC, MAGIC, AL.add, AL.subtract)
                    nc.vector.tensor_tensor(args_d[:, gsl], ud_ps[:, 0:w], i_d[:, 0:w], AL.subtract)
                    nc.vector.scalar_tensor_tensor(abs_d[:, gsl], args_d[:, gsl], -1.0, args_d[:, gsl], AL.mult, AL.max)
                with nc.named_scope(f"feat_d_{g}"):
                    nc.scalar.activation(fSd[:, gsl], args_d[:, gsl], AF.Sin, scale=TWO_PI)
                    nc.scalar.activation(fCd[:, gsl], abs_d[:, gsl], AF.Sin, bias=hp_sb[:], scale=-TWO_PI)
                with nc.named_scope(f"vfold_{g}"):
                    nc.gpsimd.tensor_tensor(fSd[:, gsl], fSd[:, gsl], vbt_sb[:, gsl], AL.mult)
                    nc.gpsimd.tensor_tensor(fCd[:, gsl], fCd[:, gsl], vbt_sb[:, gsl], AL.mult)

            def e_group(g, ks):
                w = len(ks) * S
                gsl = slice(ks[0] * S, ks[0] * S + w)
                with nc.named_scope(f"ue_mm_{g}"):
                    ue_ps = psp.tile([128, 1536], F32, tag="ps", name=f"ue{g}")
                    for kk, k in enumerate(ks):
                        nc.tensor.matmul(
                            ue_ps[:, kk * S:(kk + 1) * S],
                            we_sb[:, k * H:(k + 1) * H],
                            encT[:],
                            start=True,
                            stop=True,
                        )
                with nc.named_scope(f"round_e_{g}"):
                    i_e = wrk.tile([128, 1536], F32, tag="i_e", name=f"ie{g}")
                    nc.vector.tensor_scalar(i_e[:, 0:w], ue_ps[:, 0:w], MAGIC, MAGIC, AL.add, AL.subtract)
                    nc.vector.tensor_tensor(args_e[:, gsl], ue_ps[:, 0:w], i_e[:, 0:w], AL.subtract)
                with nc.named_scope(f"feat_e_{g}"):
                    nc.scalar.activation(abs_e[:, gsl], args_e[:, gsl], AF.Abs)
                    nc.scalar.activation(fSe[:, gsl], args_e[:, gsl], AF.Sin, scale=TWO_PI)
                    nc.scalar.activation(fCe[:, gsl], abs_e[:, gsl], AF.Sin, bias=hp_sb[:], scale=-TWO_PI)

            d_group(0, DG[0])
            e_group(0, EG[0])
            e_group(1, EG[1])
            d_group(1, DG[1])
            e_group(2, EG[2])

            # ---- scores: accumulate per (tb, e-group) so g0 matmuls start
            # before g1 features are ready; -1e30 enc mask via rank-1 matmul ----
            sc = []
            for tb in range(2):
                sc_tile = psp.tile([128, S], F32, tag="sc", name=f"sc{tb}")
                sc.append(sc_tile)
                with nc.named_scope(f"mask_{tb}"):
                    nc.tensor.matmul(
                        sc_tile[:], ones_sb[:], em_sb[:],
                        start=True, stop=False, skip_group_check=True,
                    )
            for g, ks in enumerate(EG):
                for tb in range(2):
                    with nc.named_scope(f"scores_{tb}_{g}"):
                        for k in ks:
                            dsl = slice(k * T + tb * 128, k * T + (tb + 1) * 128)
                            esl = slice(k * S, (k + 1) * S)
                            nc.tensor.matmul(
                                sc[tb][:], fSd[:, dsl], fCe[:, esl],
                                start=False, stop=False, skip_group_check=True,
                            )
                            nc.tensor.matmul(
                                sc[tb][:], fCd[:, dsl], fSe[:, esl],
                                start=False, stop=(k == F - 1), skip_group_check=True,
                            )
            for tb in range(2):
                with nc.named_scope(f"softmax_{tb}"):
                    ex = wrk.tile([128, S], F32, tag="ex")
                    rs = wrk.tile([128, 1], F32, tag="rs")
                    nc.scalar.activation(ex[:], sc[tb][:], AF.Exp, accum_out=rs[:])
                    ri = wrk.tile([128, 1], F32, tag="ri")
                    nc.vector.reciprocal(ri[:], rs[:])
                    fac = wrk.tile([128, 1], F32, tag="fac")
                    nc.vector.tensor_tensor(fac[:], ri[:], dm_sb[:, tb:tb + 1], AL.mult)
                    ot = wrk.tile([128, S], F32, tag="ot")
                    nc.vector.tensor_scalar_mul(ot[:], ex[:], fac[:])
                    nc.sync.dma_start(out_d[tb * 128:(tb + 1) * 128, :], ot[:])

    nc.compile()
    _CACHE["nc"] = nc
    return nc


def kernel(encoder_output, decoder_output, W1, W2, v, enc_lens, dec_lens):
    global LAST_EXEC_NS
    from concourse.bass_utils import run_bass_kernel_spmd

    enc = np.ascontiguousarray(np.asarray(encoder_output, dtype=np.float32))
    dec = np.ascontiguousarray(np.asarray(decoder_output, dtype=np.float32))
    W1 = np.asarray(W1, dtype=np.float32)
    W2 = np.asarray(W2, dtype=np.float32)
    v = np.asarray(v, dtype=np.float32)
    enc_lens = np.asarray(enc_lens)
    dec_lens = np.asarray(dec_lens)

    scal = (OMEGA / (2.0 * np.pi)).astype(np.float32)
    Wd = np.stack([W2 * scal[k] for k in range(F)]).astype(np.float32)
    We = np.stack([W1 * scal[k] for k in range(F)]).astype(np.float32)
    import ml_dtypes

    vb = (v[:, None].astype(np.float64) * BK[None, :]).astype(np.float32)
    vbt = np.repeat(vb, T, axis=1).astype(ml_dtypes.bfloat16)  # (H, F*T)
    eye = np.eye(H, dtype=np.float32)

    in_maps = []
    for b in range(B):
        em = np.where(np.arange(S)[None, :] < int(enc_lens[b]), 0.0, NEG_BIG).astype(np.float32)
        dm = (np.arange(T)[:, None] < int(dec_lens[b])).astype(np.float32)
        in_maps.append(
            {
                "enc": np.ascontiguousarray(enc[:, b, :]),
                "dec": np.ascontiguousarray(dec[:, b, :]),
                "Wd": Wd,
                "We": We,
                "vbt": vbt,
                "eye": eye,
                "encmask": em,
                "decmask": dm,
            }
        )

    trace = os.environ.get("KERNEL_TRACE", "0") == "1"
    if trace:
        _try_install_trace_hook()
    nc = _build()
    ncores = int(os.environ.get("KERNEL_CORES", str(B)))
    res = run_bass_kernel_spmd(nc, in_maps[:ncores], core_ids=list(range(ncores)), trace=trace)
    if trace:
        LAST_EXEC_NS = res.exec_time_ns
        _CACHE["last_res"] = res

    out = np.zeros((T, B, S), dtype=np.float32)
    for b in range(ncores):
        out[:, b, :] = res.results[b]["out"]
    return out

